# revision 25
# baseline (speedup 1.0000x reference)
"""Self-contained Trainium2 (Bass/Tile) kernel for the AttentionGRUCell
problem: 8-core data-parallel over batch, bf16 matmul operands
(host-cast), fp32 accumulation.

kernel(**inputs) takes the FULL unsharded inputs and returns the FULL
[512, 1088] output ([alpha, h_new] per row), running the Bass program on
NeuronCores 0-7 via run_bass_kernel_spmd.
"""
import sys

for _p in ("/opt/trn_rl_repo",):
    if _p not in sys.path:
        sys.path.insert(0, _p)

import numpy as np
import ml_dtypes
import concourse.bass as bass
import concourse.mybir as mybir
import concourse.tile as tile
import bass_rust
from concourse.alu_op_type import AluOpType
from concourse.masks import make_identity
from concourse.vector_clock import ScopedClock

F32 = mybir.dt.float32
BF16 = mybir.dt.bfloat16
FP8 = mybir.dt.float8e4
AF = mybir.ActivationFunctionType
AX = mybir.AxisListType

Bc, T, XD, ED, U = 64, 64, 512, 1024, 1024
NSBLK = 8
N_CORES = 8
B_FULL = 512

NP_BF16 = ml_dtypes.bfloat16
NP_FP8 = ml_dtypes.float8_e4m3
VA_SCALE = 32.0


# ---------------------------------------------------------------------------
# Workarounds for this walrus build: instructions may carry at most one sem
# wait ("Too many sync wait commands"), including the Tile kernel-tail drain.
# ---------------------------------------------------------------------------

def _patched_drain_and_barrier(self, tick_clock, wait_clock):
    nc = self.nc
    probe = nc.sync.nop(nofuse=True)
    wait_clock.add_sem_waits(probe.ins, ScopedClock({None: tick_clock.global_clock}))
    si = probe.ins.sync_info
    waits = list(si.on_wait) if si is not None else []
    probe.ins.sync_info = bass_rust.SyncInfo(on_wait=waits[:1], on_update=[])
    for w in waits[1:]:
        n2 = nc.sync.nop(nofuse=True)
        n2.ins.sync_info = bass_rust.SyncInfo(on_wait=[w], on_update=[])
    nc.sync.drain()
    nc.all_engine_barrier()
    assert self.sems is not None
    popped = nc._tile_sem_poison_stack.pop()
    assert popped is self._sem_poison
    nc.clear_and_free_semaphores(list(self.sems.allocated().values()))
    nc.all_engine_barrier()


tile.TileContext._drain_and_barrier = _patched_drain_and_barrier

_fix_ctr = [0]


def fix_multi_waits(nc, max_waits=1):
    """Hoist extra sem waits onto same-engine InstNoOps placed immediately
    before the instruction -- engines execute in order, so semantics are
    identical."""
    for f in nc.m.functions:
        for blk in f.blocks:
            insts = blk.instructions
            if not any(
                i.sync_info is not None and len(i.sync_info.on_wait) > max_waits
                for i in insts
            ):
                continue
            out = []
            for inst in insts:
                si = inst.sync_info
                if si is not None and len(si.on_wait) > max_waits:
                    waits = list(si.on_wait)
                    for w in waits[:-max_waits]:
                        _fix_ctr[0] += 1
                        nop = mybir.InstNoOp(
                            name=f"waitfix-{_fix_ctr[0]}",
                            ins=[],
                            outs=[],
                            engine=inst.engine,
                        )
                        nop.sync_info = bass_rust.SyncInfo(on_wait=[w], on_update=[])
                        out.append(nop)
                    inst.sync_info = bass_rust.SyncInfo(
                        on_wait=waits[-max_waits:], on_update=list(si.on_update)
                    )
                out.append(inst)
            blk.instructions = out


# ---------------------------------------------------------------------------
# Kernel program
# ---------------------------------------------------------------------------

def build_nc():
    nc = bass.Bass("TRN2", target_bir_lowering=False, debug=False)

    # host-prepped per-core inputs (all bf16 except h/bias):
    #   esr:  [Bc*T, ED]      natural layout, row = 64*b + t
    #   hT:   [128, 8*Bc]     hT[p, d*Bc+b] = h[b, 128d+p]
    #   inT:  [128, 4*Bc]     inT[p, d*Bc+b] = inputs[b, 128d+p]
    #   va:   [128, 8]        va[p, j] = Va[128j+p, 0]
    es_d = nc.dram_tensor("esr", [Bc * T, ED], BF16, kind="ExternalInput")
    h_d = nc.dram_tensor("h", [Bc, U], F32, kind="ExternalInput")
    # smallpack: hT [128, 512] | inT [128, 256] | va [128, 8]  (one DMA)
    sp_d = nc.dram_tensor("smallpack", [128, 8 * Bc + 4 * Bc + 8], BF16,
                          kind="ExternalInput")
    kernel_d = nc.dram_tensor("kernel", [XD + ED, 3 * U], BF16, kind="ExternalInput")
    rk_d = nc.dram_tensor("recurrent_kernel", [U, 3 * U], BF16, kind="ExternalInput")
    bias_d = nc.dram_tensor("bias", [3 * U], BF16, kind="ExternalInput")
    wat_d = nc.dram_tensor("wat", [U, U], BF16, kind="ExternalInput")
    wa8_d = nc.dram_tensor("wa8", [128, 4, 2, U], FP8, kind="ExternalInput")
    va8_d = nc.dram_tensor("va8", [128, 2, 16], FP8, kind="ExternalInput")
    out_d = nc.dram_tensor("out", [Bc, T + U], F32, kind="ExternalOutput")

    es_ap = es_d.ap()

    with tile.TileContext(nc) as tc:
        with (
            tc.tile_pool(name="singles", bufs=1) as sg,
            tc.tile_pool(name="esr", bufs=3) as esr_pool,
            tc.tile_pool(name="tesT", bufs=8) as tesT_pool,
            tc.tile_pool(name="gT", bufs=8) as gT_pool,
            tc.tile_pool(name="wat", bufs=8) as wat_pool,
            tc.tile_pool(name="ghx", bufs=2) as ghx_pool,
            tc.tile_pool(name="smalls", bufs=4) as sm_pool,
            # 8 PSUM banks total; tail reuses attention tags
            tc.tile_pool(name="ps_tr", bufs=2, space="PSUM") as ps_tr,
            tc.tile_pool(name="ps_v", bufs=2, space="PSUM") as ps_v,
            tc.tile_pool(name="ps_acc", bufs=2, space="PSUM") as ps_acc,
            tc.tile_pool(name="ps_e", bufs=1, space="PSUM") as ps_e,
            tc.tile_pool(name="ps_ct", bufs=1, space="PSUM") as ps_ct,
        ):
            # ---- earliest DMAs: es sblk0, weights ----
            def load_esr(g, eng=None):
                e_t = esr_pool.tile([128, 4, ED], BF16, tag="esr", name=f"esr{g}")
                src = es_ap.rearrange("(g r p) e -> g p r e", p=128, r=4)
                e = eng or nc.gpsimd
                e.dma_start(out=e_t[:, 0:2, :], in_=src[g, :, 0:2, :])
                e.dma_start(out=e_t[:, 2:4, :], in_=src[g, :, 2:4, :])
                return e_t

            esr_cur = load_esr(0)

            wa8_sb = sg.tile([128, 4, 2, U], FP8)
            nc.sync.dma_start(out=wa8_sb[:, 0:2], in_=wa8_d.ap()[:, 0:2])
            nc.sync.dma_start(out=wa8_sb[:, 2:4], in_=wa8_d.ap()[:, 2:4])
            va8_sb = sg.tile([128, 2, 16], FP8)
            nc.sync.dma_start(out=va8_sb[:], in_=va8_d.ap())

            sp_sb = sg.tile([128, 8 * Bc + 4 * Bc + 8], BF16)
            nc.scalar.dma_start(out=sp_sb[:], in_=sp_d.ap())
            hT_sb = sp_sb[:, 0:8 * Bc].rearrange("p (d b) -> p d b", b=Bc)
            inT_sb = sp_sb[:, 8 * Bc:12 * Bc].rearrange("p (d b) -> p d b", b=Bc)
            va_sb = sp_sb[:, 12 * Bc:12 * Bc + 8]
            h_sb = sg.tile([Bc, U], F32)
            nc.scalar.dma_start(out=h_sb[:], in_=h_d[:])
            bias_b = sg.tile([Bc, 3 * U], BF16)
            bias_src = bias_d.ap()
            nc.scalar.dma_start(
                out=bias_b[:],
                in_=bass.AP(
                    tensor=bias_src.tensor,
                    offset=bias_src.offset,
                    ap=[[0, Bc], list(bias_src.ap[0])],
                ),
            )

            # GRU kernel bottom (c_t part): DMA emitted at g==1
            kbot_sb = sg.tile([128, 8, 3 * U], BF16)

            def load_kbot():
                src = kernel_d.ap()[XD:, :].rearrange("(d p) u -> p d u", p=128)
                nc.scalar.dma_start(out=kbot_sb[:, 0:3], in_=src[:, 0:3])
                nc.scalar.dma_start(out=kbot_sb[:, 3:6], in_=src[:, 3:6])
                nc.scalar.dma_start(out=kbot_sb[:, 6:8], in_=src[:, 6:8])

            # Wa top half (h part), for qk -- all 8 chunks up front (scalar q)
            def load_wat(d):
                w_t = wat_pool.tile([128, U], BF16, tag="wat", name=f"wat{d}")
                nc.scalar.dma_start(out=w_t[:], in_=wat_d.ap()[128 * d:128 * (d + 1), :])
                return w_t

            wat_tiles = [load_wat(d) for d in range(8)]

            esr_nxt = load_esr(1)

            ident = sg.tile([128, 128], F32)
            make_identity(nc, ident[:])
            identB = sg.tile([128, 128], BF16)
            nc.vector.tensor_copy(identB[:], ident[:])

            # thT = tanh(h).T directly in transposed layout
            thT = sg.tile([128, 8, Bc], BF16)
            nc.scalar.activation(out=thT[:], in_=hT_sb, func=AF.Tanh)

            # masks for block-diag A build
            masks = sg.tile([128, 4, 8], F32)
            nc.vector.memset(masks[:], 0.0)
            for rr in range(4):
                nc.vector.memset(masks[0:64, rr, 2 * rr:2 * rr + 1], 1.0)
                nc.vector.memset(masks[64:128, rr, 2 * rr + 1:2 * rr + 2], 1.0)

            half_sb = sg.tile([Bc, 1], F32)
            nc.vector.memset(half_sb[:], 0.5)

            ct_sb = sg.tile([Bc, ED], BF16)
            gh_sb = sg.tile([Bc, 2 * U], BF16)
            gx0_sb = sg.tile([Bc, 3 * U], BF16)
            qk_nat = sg.tile([Bc, U], F32)
            qkT = sg.tile([128, 8, Bc], BF16)

            # GRU recurrent_kernel hh columns (DMA emitted mid-loop)
            rkh_sb = sg.tile([128, 8, U], BF16)

            _tp_ctr = [0]

            def transpose_to(dst, src_2d, j, idt, dt):
                _tp_ctr[0] += 1
                pt = ps_tr.tile([128, Bc], dt, tag="tr", name=f"tp{_tp_ctr[0]}")
                nc.tensor.transpose(pt[:], src_2d, idt[:Bc, :Bc])
                nc.vector.tensor_copy(dst[:, j, :], pt[:])

            # ---- per-sblk building blocks ----
            def emit_tesT(g, esr_g):
                # DoubleRow pairs: tesT[jp][p, s, bt] = tanh(es.T)[256jp+128s+p, bt]
                tesT = []
                for jp in range(4):
                    pt = ps_tr.tile([128, 2, 512], BF16, tag="tr", name=f"ptr{g}_{jp}")
                    for jj in range(2):
                        for rr in range(4):
                            nc.tensor.transpose(
                                pt[:, jj, 128 * rr:128 * (rr + 1)],
                                esr_g[:, rr, 128 * (2 * jp + jj):128 * (2 * jp + jj + 1)],
                                identB[:],
                            )
                    tt = tesT_pool.tile([128, 2, 512], FP8, tag="tesT", name=f"tes{g}_{jp}")
                    nc.scalar.activation(out=tt[:], in_=pt[:], func=AF.Tanh)
                    tesT.append(tt)
                return tesT

            def emit_stt_gt(g, c, pv, gtp):
                # gtp[:, c%2, :] = pv + qk (broadcast over t)
                qk_slice = qkT[:, c, 8 * g:8 * g + 8]
                qk_bc = bass.AP(
                    tensor=qk_slice.tensor,
                    offset=qk_slice.offset,
                    ap=[
                        list(qk_slice.ap[0]),
                        list(qk_slice.ap[1]),
                        [0, T],
                    ],
                )
                nc.vector.scalar_tensor_tensor(
                    out=gtp[:, c % 2, :],
                    in0=pv[:],
                    scalar=1.0,
                    in1=qk_bc,
                    op0=AluOpType.mult,
                    op1=AluOpType.add,
                )

            def emit_gt8(g, cp, gtp):
                gt8 = gT_pool.tile([128, 2, 512], FP8, tag="gT", name=f"g8_{g}_{cp}")
                nc.scalar.activation(out=gt8[:], in_=gtp[:], func=AF.Tanh)
                return gt8

            def ct_head(g, alpha):
                pat = ps_ct.tile([64, 8], F32, tag="ct", name=f"pat{g}")
                nc.tensor.transpose(pat[:], alpha[:], ident[:8, :8])
                alpT2 = sm_pool.tile([128, 8], F32, tag="alT2", name=f"aT2{g}")
                nc.vector.tensor_copy(alpT2[0:64, :], pat[:])
                nc.gpsimd.dma_start(out=alpT2[64:128, :], in_=alpT2[0:64, :])
                ars = []
                for rr in range(4):
                    a_r = sm_pool.tile([128, 8], BF16, tag="A", name=f"A{g}_{rr}")
                    nc.gpsimd.tensor_mul(a_r[:], alpT2[:], masks[:, rr, :])
                    ars.append(a_r)
                return ars

            def ct_tail(g, esr_g, ars):
                ct_stage = sm_pool.tile(
                    [8, ED], BF16, tag="ctst", name=f"cts{g}", bufs=1
                )
                for n in range(2):
                    pct = ps_ct.tile([8, 512], F32, tag="ct", name=f"pct{g}_{n}")
                    for rr in range(4):
                        nc.tensor.matmul(
                            pct[:],
                            ars[rr][:],
                            esr_g[:, rr, 512 * n:512 * (n + 1)],
                            start=(rr == 0),
                            stop=(rr == 3),
                        )
                    nc.vector.tensor_copy(ct_stage[:, 512 * n:512 * (n + 1)], pct[:])
                nc.gpsimd.dma_start(out=ct_sb[8 * g:8 * (g + 1), :], in_=ct_stage[:])

            def gh_block(n, ch0, ch1):
                # gh[:, n] = h @ rk[:, n-slice] + bias[n-slice]
                pg = ps_acc.tile([Bc, 512], F32, tag="acc", name=f"pg{n}")
                for d in range(8):
                    ch = ch0 if d < 4 else ch1
                    nc.tensor.matmul(
                        pg[:], hT_sb[:, d, :], ch[:, d % 4, :],
                        start=(d == 0), stop=(d == 7),
                    )
                nc.vector.scalar_tensor_tensor(
                    out=gh_sb[:, 512 * n:512 * (n + 1)],
                    in0=pg[:],
                    scalar=1.0,
                    in1=bias_b[:, 512 * n:512 * (n + 1)],
                    op0=AluOpType.mult,
                    op1=AluOpType.add,
                )

            def gx0_block(n, chunk):
                # inputs-part of the x @ kernel gates
                pa = ps_acc.tile([Bc, 512], F32, tag="acc", name=f"gx0_{n}")
                for d in range(4):
                    nc.tensor.matmul(
                        pa[:], inT_sb[:, d, :], chunk[:, d, :],
                        start=(d == 0), stop=(d == 3),
                    )
                nc.vector.tensor_copy(gx0_sb[:, 512 * n:512 * (n + 1)], pa[:])

            def load_gh_chunk(n, half):
                ch = ghx_pool.tile([128, 4, 512], BF16, tag="ghx", name=f"rkc{n}_{half}")
                src = rk_d.ap().rearrange("(d p) u -> p d u", p=128)
                nc.sync.dma_start(
                    out=ch[:],
                    in_=src[:, 4 * half:4 * (half + 1), 512 * n:512 * (n + 1)],
                )
                return ch

            def load_gx0_chunk(n):
                ch = ghx_pool.tile([128, 4, 512], BF16, tag="ghx", name=f"knc{n}")
                src = kernel_d.ap().rearrange("(d p) u -> p d u", p=128)
                nc.sync.dma_start(
                    out=ch[:], in_=src[:, 0:4, 512 * n:512 * (n + 1)]
                )
                return ch

            def load_rkh():
                src = rk_d.ap().rearrange("(d p) u -> p d u", p=128)
                nc.scalar.dma_start(out=rkh_sb[:, 0:4], in_=src[:, 0:4, 2 * U:])
                nc.scalar.dma_start(out=rkh_sb[:, 4:8], in_=src[:, 4:8, 2 * U:])

            # softmax over t (|e| <~ 1.5: exp w/o max-sub is safe in fp32)
            def emit_softmax(g, pe):
                e_sb = sm_pool.tile([1, 512], F32, tag="esb", name=f"esb{g}", bufs=1)
                nc.vector.tensor_copy(e_sb[:], pe[:])
                alpha = sm_pool.tile([8, T], F32, tag="al", name=f"al{g}")
                nc.gpsimd.dma_start(
                    out=alpha[:],
                    in_=e_sb[0:1, :].rearrange("p (b t) -> p b t", b=8),
                )
                ssum = sm_pool.tile([8, 1], F32, tag="ssum", name=f"ss{g}")
                nc.scalar.activation(
                    out=alpha[:], in_=alpha[:], func=AF.Exp,
                    scale=1.0 / VA_SCALE, accum_out=ssum[:]
                )
                srec = sm_pool.tile([8, 1], F32, tag="srec", name=f"sr{g}")
                nc.vector.reciprocal(srec[:], ssum[:])
                nc.gpsimd.tensor_scalar_mul(alpha[:], alpha[:], srec[:])
                nc.gpsimd.dma_start(
                    out=out_d.ap()[8 * g:8 * (g + 1), 0:T], in_=alpha[:]
                )
                return alpha

            # =================== superblock 0 (j-outer pv) ===================
            tesT = emit_tesT(0, esr_cur)

            # all eight psum banks accumulate u-chunks 0..7 across the j
            # stream so the PE can start as soon as the first wab chunk lands
            pv_banks = [
                ps_v.tile([128, 512], F32, tag="v", name="pvv0"),
                ps_v.tile([128, 512], F32, tag="v", name="pvv1"),
                ps_acc.tile([128, 512], F32, tag="acc", name="pva0"),
                ps_acc.tile([128, 512], F32, tag="acc", name="pva1"),
                ps_e.tile([128, 512], F32, tag="e", name="pve"),
                ps_ct.tile([128, 512], F32, tag="ct", name="pvc"),
                ps_tr.tile([128, 512], F32, tag="tr", name="pvt0"),
                ps_tr.tile([128, 512], F32, tag="tr", name="pvt1"),
            ]
            for jp in range(4):
                for c in range(8):
                    nc.tensor.matmul(
                        pv_banks[c][:],
                        wa8_sb[:, jp, :, 128 * c:128 * (c + 1)],
                        tesT[jp][:],
                        start=(jp == 0),
                        stop=(jp == 3),
                        perf_mode=mybir.MatmulPerfMode.DoubleRow,
                    )
            # raw (pre-qk) gate values to SBUF pairs; frees all banks quickly
            gtps = []
            for cp in range(4):
                gtp = sm_pool.tile([128, 2, 512], BF16, tag="gtp", name=f"gtp0_{cp}")
                gtps.append(gtp)
            for c in range(8):
                nc.vector.tensor_copy(gtps[c // 2][:, c % 2, :], pv_banks[c][:])

            # next sblk's transposes early (tr banks now free)
            tesT_nxt = emit_tesT(1, esr_nxt)

            # qk = tanh(h) @ Wa_top
            pqs = [
                ps_tr.tile([Bc, 512], F32, tag="tr", name=f"pq{hh}")
                for hh in range(2)
            ]
            for d in range(8):
                wat = wat_tiles[d]
                for half in range(2):
                    nc.tensor.matmul(
                        pqs[half][:],
                        thT[:, d, :],
                        wat[:, 512 * half:512 * (half + 1)],
                        start=(d == 0),
                        stop=(d == 7),
                    )
            for half in range(2):
                nc.vector.tensor_copy(qk_nat[:, 512 * half:512 * (half + 1)], pqs[half][:])
            for j in range(8):
                transpose_to(qkT, qk_nat[:, 128 * j:128 * (j + 1)], j, ident, F32)

            # add qk in place (sbuf), then fused pair tanh -> fp8
            for c in range(8):
                qk_slice = qkT[:, c, 0:8]
                qk_bc = bass.AP(
                    tensor=qk_slice.tensor,
                    offset=qk_slice.offset,
                    ap=[list(qk_slice.ap[0]), list(qk_slice.ap[1]), [0, T]],
                )
                gtp = gtps[c // 2]
                nc.vector.scalar_tensor_tensor(
                    out=gtp[:, c % 2, :], in0=gtp[:, c % 2, :], scalar=1.0,
                    in1=qk_bc, op0=AluOpType.mult, op1=AluOpType.add,
                )
            gt8s = [emit_gt8(0, cp, gtps[cp]) for cp in range(4)]

            pe = ps_e.tile([1, 512], F32, tag="e", name="pe0")
            for cp in range(4):
                nc.tensor.matmul(
                    pe[:], va8_sb[:, :, cp:cp + 1], gt8s[cp][:],
                    start=(cp == 0), stop=(cp == 3),
                    perf_mode=mybir.MatmulPerfMode.DoubleRow,
                )
            alpha = emit_softmax(0, pe)

            prev = (0, esr_cur, alpha)
            esr_cur = esr_nxt
            esr_nxt = None  # loaded inside the loop

            # =================== superblocks 1..7 ===================
            gh_chunks = (load_gh_chunk(0, 0), load_gh_chunk(0, 1))
            for g in range(1, NSBLK):
                tesT = tesT_nxt if g == 1 else emit_tesT(g, esr_cur)

                ars_prev = None
                gt8s = []
                gtp = None
                for c in range(8):
                    pv = ps_v.tile([128, 512], F32, tag="v", name=f"pv{g}_{c}")
                    for jp in range(4):
                        nc.tensor.matmul(
                            pv[:],
                            wa8_sb[:, jp, :, 128 * c:128 * (c + 1)],
                            tesT[jp][:],
                            start=(jp == 0),
                            stop=(jp == 3),
                            perf_mode=mybir.MatmulPerfMode.DoubleRow,
                        )
                    if c % 2 == 0:
                        gtp = sm_pool.tile(
                            [128, 2, 512], BF16, tag="gtp", name=f"gtp{g}_{c // 2}"
                        )
                    emit_stt_gt(g, c, pv, gtp)
                    if c % 2 == 1:
                        gt8s.append(emit_gt8(g, c // 2, gtp))
                    if c == 1:
                        ars_prev = ct_head(prev[0], prev[2])
                    if c == 5:
                        ct_tail(prev[0], prev[1], ars_prev)
                    if c == 6 and g + 1 < NSBLK:
                        esr_nxt2 = load_esr(g + 1)

                pe = ps_e.tile([1, 512], F32, tag="e", name=f"pe{g}")
                for cp in range(4):
                    nc.tensor.matmul(
                        pe[:], va8_sb[:, :, cp:cp + 1], gt8s[cp][:],
                        start=(cp == 0), stop=(cp == 3),
                        perf_mode=mybir.MatmulPerfMode.DoubleRow,
                    )

                # spread GRU weight streams across the attention phase
                if 1 <= g <= 4:
                    gh_block(g - 1, *gh_chunks)
                    if g < 4:
                        gh_chunks = (load_gh_chunk(g, 0), load_gh_chunk(g, 1))
                    else:
                        gx0_chunks = (load_gx0_chunk(0), load_gx0_chunk(1))
                if g == 3:
                    load_kbot()
                if g == 5:
                    load_rkh()
                if 5 <= g <= 7:
                    n0 = 2 * (g - 5)
                    gx0_block(n0, gx0_chunks[0])
                    gx0_block(n0 + 1, gx0_chunks[1])
                    if g < 7:
                        gx0_chunks = (load_gx0_chunk(n0 + 2), load_gx0_chunk(n0 + 3))

                alpha = emit_softmax(g, pe)

                if g == 7:
                    # pre-sum the z/r and hh additive terms for the tail
                    gzr_sb = sg.tile([Bc, 2 * U], BF16)
                    for n in range(4):
                        nc.vector.tensor_add(
                            gzr_sb[:, 512 * n:512 * (n + 1)],
                            gx0_sb[:, 512 * n:512 * (n + 1)],
                            gh_sb[:, 512 * n:512 * (n + 1)],
                        )
                    hsum_sb = sg.tile([Bc, U], BF16)
                    for n in range(2):
                        nc.vector.tensor_add(
                            hsum_sb[:, 512 * n:512 * (n + 1)],
                            gx0_sb[:, 2 * U + 512 * n:2 * U + 512 * (n + 1)],
                            bias_b[:, 2 * U + 512 * n:2 * U + 512 * (n + 1)],
                        )

                prev = (g, esr_cur, alpha)
                esr_cur = esr_nxt2 if g + 1 < NSBLK else None

            ars_prev = ct_head(prev[0], prev[2])
            ct_tail(prev[0], prev[1], ars_prev)

            # PE warm-keeper: dependency-free matmuls hold the PE pstate up
            # while the last alpha -> c_t -> ctT chain runs on other engines
            for w in range(2):
                pwarm = ps_v.tile([128, 512], F32, tag="v", name=f"warm{w}")
                for k in range(20):
                    nc.tensor.matmul(
                        pwarm[:],
                        identB[:],
                        kbot_sb[:, 0, 0:512],
                        start=(k == 0),
                        stop=(k == 19),
                    )

            # =================== GRU tail ===================
            ctT = sg.tile([128, 8, Bc], BF16)
            for j in range(8):
                transpose_to(ctT, ct_sb[:, 128 * j:128 * (j + 1)], j, identB, BF16)

            z_sb = sg.tile([Bc, U], F32)
            r_sb = sg.tile([Bc, U], F32, tag="scr_r_t1")
            hh_sb = sg.tile([Bc, U], F32)
            rh_sb = sg.tile([Bc, U], BF16)
            rhT = sg.tile([128, 8, Bc], BF16)
            t1 = None  # allocated after r is consumed (shares r's buffer)

            # six gate accumulators in the (now idle) attention psum banks
            gx = [
                ps_tr.tile([Bc, 512], F32, tag="tr", name="gxa"),
                ps_tr.tile([Bc, 512], F32, tag="tr", name="gxb"),
                ps_acc.tile([Bc, 512], F32, tag="acc", name="gxc"),
                ps_acc.tile([Bc, 512], F32, tag="acc", name="gxd"),
                ps_e.tile([Bc, 512], F32, tag="e", name="gxe"),
                ps_ct.tile([Bc, 512], F32, tag="ct", name="gxf"),
            ]
            # pass 1: z/r gate columns (n 0..3) so the gate math starts early
            for d in range(8):
                for n in range(4):
                    nc.tensor.matmul(
                        gx[n][:],
                        ctT[:, d, :],
                        kbot_sb[:, d, 512 * n:512 * (n + 1)],
                        start=(d == 0),
                        stop=(d == 7),
                    )
            # pass 2: hh gate columns (n 4,5); kept open for the rkh stream
            for d in range(8):
                for n in range(4, 6):
                    nc.tensor.matmul(
                        gx[n][:],
                        ctT[:, d, :],
                        kbot_sb[:, d, 512 * n:512 * (n + 1)],
                        start=(d == 0),
                        stop=False,
                    )

            def add_inplace(pa, src_sb, o):
                nc.vector.scalar_tensor_tensor(
                    out=pa[:],
                    in0=pa[:],
                    scalar=1.0,
                    in1=src_sb[:, o:o + 512],
                    op0=AluOpType.mult,
                    op1=AluOpType.add,
                )

            # z, r gates: hard_sigmoid(x) = min(relu(0.2x+0.5), 1)
            for n in range(4):
                dst = z_sb if n < 2 else r_sb
                o = 512 * (n % 2)
                add_inplace(gx[n], gzr_sb, 512 * n)
                nc.scalar.activation(
                    out=dst[:, o:o + 512], in_=gx[n][:],
                    func=AF.Relu, bias=half_sb[:], scale=0.2,
                )
                nc.vector.tensor_scalar_min(dst[:, o:o + 512], dst[:, o:o + 512], 1.0)
                if n >= 2:
                    nc.vector.tensor_mul(
                        rh_sb[:, o:o + 512], r_sb[:, o:o + 512], h_sb[:, o:o + 512]
                    )
                    for j in range(4 * (n - 2), 4 * (n - 1)):
                        transpose_to(rhT, rh_sb[:, 128 * j:128 * (j + 1)], j, identB, BF16)

            # hh accumulators continue with the (r*h) @ rk_hh stream
            for d in range(8):
                for n2 in range(2):
                    nc.tensor.matmul(
                        gx[4 + n2][:],
                        rhT[:, d, :],
                        rkh_sb[:, d, 512 * n2:512 * (n2 + 1)],
                        start=False,
                        stop=(d == 7),
                    )

            # hh = tanh(gates_hh + hsum); h_new = hh + z*(h - hh)
            t1 = sg.tile([Bc, U], F32, tag="scr_r_t1")
            for n2 in range(2):
                o = 512 * n2
                sl = slice(o, o + 512)
                pa = gx[4 + n2]
                add_inplace(pa, hsum_sb, o)
                nc.scalar.activation(out=hh_sb[:, sl], in_=pa[:], func=AF.Tanh)
                nc.vector.tensor_sub(t1[:, sl], h_sb[:, sl], hh_sb[:, sl])
                nc.vector.tensor_mul(t1[:, sl], z_sb[:, sl], t1[:, sl])
                nc.vector.tensor_add(t1[:, sl], hh_sb[:, sl], t1[:, sl])
                nc.sync.dma_start(out=out_d.ap()[:, T + o:T + o + 512], in_=t1[:, sl])

    return nc


_built = [None]


def _to_bf16(x):
    return np.ascontiguousarray(np.asarray(x, dtype=np.float32)).astype(NP_BF16)


def _transpose_chunks(x, nd):
    # x [Bc, nd*128] f32 -> [128, nd*Bc] bf16 with out[p, d*Bc+b] = x[b, 128d+p]
    b, _ = x.shape
    xt = np.ascontiguousarray(x.T).reshape(nd, 128, b).transpose(1, 0, 2)
    return np.ascontiguousarray(xt.reshape(128, nd * b)).astype(NP_BF16)


def make_in_maps(inputs):
    def f32(name):
        return np.ascontiguousarray(np.asarray(inputs[name], dtype=np.float32))

    inp = f32("inputs")
    h = f32("h")
    es = f32("encoder_states")
    ker_b = _to_bf16(inputs["kernel"])
    rk_b = _to_bf16(inputs["recurrent_kernel"])
    bias_b = _to_bf16(inputs["bias"])
    wa = np.ascontiguousarray(np.asarray(inputs["Wa"], dtype=np.float32))
    wat_b = wa[:U].astype(NP_BF16)
    wa8 = np.ascontiguousarray(
        wa[U:].reshape(4, 2, 128, U).transpose(2, 0, 1, 3)
    ).astype(NP_FP8)
    va = np.asarray(inputs["Va"], dtype=np.float32)
    va_b = np.ascontiguousarray(va[:, 0].reshape(8, 128).T).astype(NP_BF16)
    va8 = np.zeros((128, 2, 16), dtype=NP_FP8)
    va8[:, :, 0:4] = np.ascontiguousarray(
        (VA_SCALE * va[:, 0]).reshape(4, 2, 128).transpose(2, 1, 0)
    ).astype(NP_FP8)

    in_maps = []
    for c in range(N_CORES):
        sl = slice(c * Bc, (c + 1) * Bc)
        sp = np.concatenate(
            [_transpose_chunks(h[sl], 8), _transpose_chunks(inp[sl], 4), va_b],
            axis=1,
        )
        in_maps.append({
            "esr": _to_bf16(es[sl].reshape(Bc * T, ED)),
            "h": h[sl],
            "smallpack": np.ascontiguousarray(sp),
            "kernel": ker_b,
            "recurrent_kernel": rk_b,
            "bias": bias_b,
            "wat": wat_b,
            "wa8": wa8,
            "va8": va8,
        })
    return in_maps


def kernel(**inputs):
    if _built[0] is None:
        nc = build_nc()
        fix_multi_waits(nc)
        _built[0] = nc
    nc = _built[0]

    from concourse.bass_utils import run_bass_kernel_spmd

    in_maps = make_in_maps(inputs)
    res = run_bass_kernel_spmd(nc, in_maps, list(range(N_CORES)))
    out = np.concatenate(
        [res.results[c]["out"] for c in range(N_CORES)], axis=0
    ).astype(np.float32)
    return out


# revision 28
# speedup vs baseline: 1.0763x; 1.0763x over previous
"""Self-contained Trainium2 (Bass/Tile) kernel for the AttentionGRUCell
problem: 8-core data-parallel over batch, bf16 matmul operands
(host-cast), fp32 accumulation.

kernel(**inputs) takes the FULL unsharded inputs and returns the FULL
[512, 1088] output ([alpha, h_new] per row), running the Bass program on
NeuronCores 0-7 via run_bass_kernel_spmd.
"""
import sys

for _p in ("/opt/trn_rl_repo",):
    if _p not in sys.path:
        sys.path.insert(0, _p)

import numpy as np
import ml_dtypes
import concourse.bass as bass
import concourse.mybir as mybir
import concourse.tile as tile
import bass_rust
from concourse.alu_op_type import AluOpType
from concourse.masks import make_identity
from concourse.vector_clock import ScopedClock

F32 = mybir.dt.float32
BF16 = mybir.dt.bfloat16
FP8 = mybir.dt.float8e4
AF = mybir.ActivationFunctionType
AX = mybir.AxisListType

Bc, T, XD, ED, U = 64, 64, 512, 1024, 1024
NSBLK = 8
N_CORES = 8
B_FULL = 512

NP_BF16 = ml_dtypes.bfloat16
NP_FP8 = ml_dtypes.float8_e4m3
VA_SCALE = 32.0


# ---------------------------------------------------------------------------
# Workarounds for this walrus build: instructions may carry at most one sem
# wait ("Too many sync wait commands"), including the Tile kernel-tail drain.
# ---------------------------------------------------------------------------

def _patched_drain_and_barrier(self, tick_clock, wait_clock):
    nc = self.nc
    probe = nc.sync.nop(nofuse=True)
    wait_clock.add_sem_waits(probe.ins, ScopedClock({None: tick_clock.global_clock}))
    si = probe.ins.sync_info
    waits = list(si.on_wait) if si is not None else []
    probe.ins.sync_info = bass_rust.SyncInfo(on_wait=waits[:1], on_update=[])
    for w in waits[1:]:
        n2 = nc.sync.nop(nofuse=True)
        n2.ins.sync_info = bass_rust.SyncInfo(on_wait=[w], on_update=[])
    nc.sync.drain()
    nc.all_engine_barrier()
    assert self.sems is not None
    popped = nc._tile_sem_poison_stack.pop()
    assert popped is self._sem_poison
    nc.clear_and_free_semaphores(list(self.sems.allocated().values()))
    nc.all_engine_barrier()


tile.TileContext._drain_and_barrier = _patched_drain_and_barrier

_fix_ctr = [0]


def fix_multi_waits(nc, max_waits=1):
    """Hoist extra sem waits onto same-engine InstNoOps placed immediately
    before the instruction -- engines execute in order, so semantics are
    identical."""
    for f in nc.m.functions:
        for blk in f.blocks:
            insts = blk.instructions
            if not any(
                i.sync_info is not None and len(i.sync_info.on_wait) > max_waits
                for i in insts
            ):
                continue
            out = []
            for inst in insts:
                si = inst.sync_info
                if si is not None and len(si.on_wait) > max_waits:
                    waits = list(si.on_wait)
                    for w in waits[:-max_waits]:
                        _fix_ctr[0] += 1
                        nop = mybir.InstNoOp(
                            name=f"waitfix-{_fix_ctr[0]}",
                            ins=[],
                            outs=[],
                            engine=inst.engine,
                        )
                        nop.sync_info = bass_rust.SyncInfo(on_wait=[w], on_update=[])
                        out.append(nop)
                    inst.sync_info = bass_rust.SyncInfo(
                        on_wait=waits[-max_waits:], on_update=list(si.on_update)
                    )
                out.append(inst)
            blk.instructions = out


# ---------------------------------------------------------------------------
# Kernel program
# ---------------------------------------------------------------------------

def build_nc():
    nc = bass.Bass("TRN2", target_bir_lowering=False, debug=False)

    # host-prepped per-core inputs (all bf16 except h/bias):
    #   esr:  [Bc*T, ED]      natural layout, row = 64*b + t
    #   hT:   [128, 8*Bc]     hT[p, d*Bc+b] = h[b, 128d+p]
    #   inT:  [128, 4*Bc]     inT[p, d*Bc+b] = inputs[b, 128d+p]
    #   va:   [128, 8]        va[p, j] = Va[128j+p, 0]
    es_d = nc.dram_tensor("esr", [Bc * T, ED], BF16, kind="ExternalInput")
    h_d = nc.dram_tensor("h", [Bc, U], F32, kind="ExternalInput")
    # smallpack: hT [128, 512] | inT [128, 256] | va [128, 8]  (one DMA)
    sp_d = nc.dram_tensor("smallpack", [128, 8 * Bc + 4 * Bc + 8], BF16,
                          kind="ExternalInput")
    kernel_d = nc.dram_tensor("kernel", [XD + ED, 3 * U], BF16, kind="ExternalInput")
    rk_d = nc.dram_tensor("recurrent_kernel", [U, 3 * U], BF16, kind="ExternalInput")
    bias_d = nc.dram_tensor("bias", [3 * U], BF16, kind="ExternalInput")
    wat_d = nc.dram_tensor("wat", [U, U], BF16, kind="ExternalInput")
    wa8_d = nc.dram_tensor("wa8", [128, 4, 2, U], FP8, kind="ExternalInput")
    va8_d = nc.dram_tensor("va8", [128, 2, 16], FP8, kind="ExternalInput")
    out_d = nc.dram_tensor("out", [Bc, T + U], F32, kind="ExternalOutput")

    es_ap = es_d.ap()

    with tile.TileContext(nc) as tc:
        with (
            tc.tile_pool(name="singles", bufs=1) as sg,
            tc.tile_pool(name="esr", bufs=3) as esr_pool,
            tc.tile_pool(name="tesT", bufs=8) as tesT_pool,
            tc.tile_pool(name="gT", bufs=8) as gT_pool,
            tc.tile_pool(name="wat", bufs=8) as wat_pool,
            tc.tile_pool(name="ghx", bufs=2) as ghx_pool,
            tc.tile_pool(name="smalls", bufs=4) as sm_pool,
            # 8 PSUM banks total; tail reuses attention tags
            tc.tile_pool(name="ps_tr", bufs=2, space="PSUM") as ps_tr,
            tc.tile_pool(name="ps_v", bufs=2, space="PSUM") as ps_v,
            tc.tile_pool(name="ps_acc", bufs=2, space="PSUM") as ps_acc,
            tc.tile_pool(name="ps_e", bufs=1, space="PSUM") as ps_e,
            tc.tile_pool(name="ps_ct", bufs=1, space="PSUM") as ps_ct,
        ):
            # ---- earliest DMAs: es sblk0, weights ----
            def load_esr(g, eng=None):
                e_t = esr_pool.tile([128, 4, ED], BF16, tag="esr", name=f"esr{g}")
                src = es_ap.rearrange("(g r p) e -> g p r e", p=128, r=4)
                e = eng or nc.gpsimd
                e.dma_start(out=e_t[:, 0:2, :], in_=src[g, :, 0:2, :])
                e.dma_start(out=e_t[:, 2:4, :], in_=src[g, :, 2:4, :])
                return e_t

            # identity first: it feeds the PE warm-up transposes below and
            # must not queue behind DMA issues
            ident = sg.tile([128, 128], F32)
            make_identity(nc, ident[:])
            identB = sg.tile([128, 128], BF16)
            nc.vector.tensor_copy(identB[:], ident[:])

            esr_cur = load_esr(0, eng=nc.sync)

            # PE warm-up: dependency-free transposes ramp the PE pstate while
            # the first es/weight DMAs land
            for w in range(60):
                pw = ps_tr.tile([128, 128], BF16, tag="tr", name=f"wu{w}")
                nc.tensor.transpose(pw[:], identB[:], identB[:])

            wa8_sb = sg.tile([128, 4, 2, U], FP8)
            nc.sync.dma_start(out=wa8_sb[:, 0:2], in_=wa8_d.ap()[:, 0:2])
            nc.sync.dma_start(out=wa8_sb[:, 2:4], in_=wa8_d.ap()[:, 2:4])
            va8_sb = sg.tile([128, 2, 16], FP8)
            nc.sync.dma_start(out=va8_sb[:], in_=va8_d.ap())

            sp_sb = sg.tile([128, 8 * Bc + 4 * Bc + 8], BF16)
            nc.scalar.dma_start(out=sp_sb[:], in_=sp_d.ap())
            hT_sb = sp_sb[:, 0:8 * Bc].rearrange("p (d b) -> p d b", b=Bc)
            inT_sb = sp_sb[:, 8 * Bc:12 * Bc].rearrange("p (d b) -> p d b", b=Bc)
            va_sb = sp_sb[:, 12 * Bc:12 * Bc + 8]
            h_sb = sg.tile([Bc, U], F32)
            nc.scalar.dma_start(out=h_sb[:], in_=h_d[:])
            bias_b = sg.tile([Bc, 3 * U], BF16)
            bias_src = bias_d.ap()
            nc.scalar.dma_start(
                out=bias_b[:],
                in_=bass.AP(
                    tensor=bias_src.tensor,
                    offset=bias_src.offset,
                    ap=[[0, Bc], list(bias_src.ap[0])],
                ),
            )

            # GRU kernel bottom (c_t part): DMA emitted at g==1
            kbot_sb = sg.tile([128, 8, 3 * U], BF16)

            def load_kbot(d0, d1):
                src = kernel_d.ap()[XD:, :].rearrange("(d p) u -> p d u", p=128)
                nc.scalar.dma_start(out=kbot_sb[:, d0:d1], in_=src[:, d0:d1])

            # Wa top half (h part), for qk -- all 8 chunks up front (scalar q)
            def load_wat(d):
                w_t = wat_pool.tile([128, U], BF16, tag="wat", name=f"wat{d}")
                nc.scalar.dma_start(out=w_t[:], in_=wat_d.ap()[128 * d:128 * (d + 1), :])
                return w_t

            wat_tiles = [load_wat(d) for d in range(8)]

            esr_nxt = load_esr(1, eng=nc.sync)

            # thT = tanh(h).T directly in transposed layout
            thT = sg.tile([128, 8, Bc], BF16)
            nc.scalar.activation(out=thT[:], in_=hT_sb, func=AF.Tanh)

            # masks for block-diag A build
            masks = sg.tile([128, 4, 8], F32)
            nc.vector.memset(masks[:], 0.0)
            for rr in range(4):
                nc.vector.memset(masks[0:64, rr, 2 * rr:2 * rr + 1], 1.0)
                nc.vector.memset(masks[64:128, rr, 2 * rr + 1:2 * rr + 2], 1.0)

            half_sb = sg.tile([Bc, 1], F32)
            nc.vector.memset(half_sb[:], 0.5)

            ct_sb = sg.tile([Bc, ED], BF16)
            gh_sb = sg.tile([Bc, 2 * U], BF16)
            gx0_sb = sg.tile([Bc, 3 * U], BF16)
            qk_nat = sg.tile([Bc, U], F32)
            qkT = sg.tile([128, 8, Bc], BF16)

            # GRU recurrent_kernel hh columns (DMA emitted mid-loop)
            rkh_sb = sg.tile([128, 8, U], BF16)

            _tp_ctr = [0]

            def transpose_to(dst, src_2d, j, idt, dt):
                _tp_ctr[0] += 1
                pt = ps_tr.tile([128, Bc], dt, tag="tr", name=f"tp{_tp_ctr[0]}")
                nc.tensor.transpose(pt[:], src_2d, idt[:Bc, :Bc])
                nc.vector.tensor_copy(dst[:, j, :], pt[:])

            # ---- per-sblk building blocks ----
            def emit_tesT(g, esr_g):
                # DoubleRow pairs: tesT[jp][p, s, bt] = tanh(es.T)[256jp+128s+p, bt]
                tesT = []
                for jp in range(4):
                    pt = ps_tr.tile([128, 2, 512], BF16, tag="tr", name=f"ptr{g}_{jp}")
                    for jj in range(2):
                        for rr in range(4):
                            nc.tensor.transpose(
                                pt[:, jj, 128 * rr:128 * (rr + 1)],
                                esr_g[:, rr, 128 * (2 * jp + jj):128 * (2 * jp + jj + 1)],
                                identB[:],
                            )
                    tt = tesT_pool.tile([128, 2, 512], FP8, tag="tesT", name=f"tes{g}_{jp}")
                    nc.scalar.activation(out=tt[:], in_=pt[:], func=AF.Tanh)
                    tesT.append(tt)
                return tesT

            def emit_stt_gt(g, c, pv, gtp):
                # gtp[:, c%2, :] = pv + qk (broadcast over t)
                qk_slice = qkT[:, c, 8 * g:8 * g + 8]
                qk_bc = bass.AP(
                    tensor=qk_slice.tensor,
                    offset=qk_slice.offset,
                    ap=[
                        list(qk_slice.ap[0]),
                        list(qk_slice.ap[1]),
                        [0, T],
                    ],
                )
                nc.vector.scalar_tensor_tensor(
                    out=gtp[:, c % 2, :],
                    in0=pv[:],
                    scalar=1.0,
                    in1=qk_bc,
                    op0=AluOpType.mult,
                    op1=AluOpType.add,
                )

            def emit_gt8(g, cp, gtp):
                gt8 = gT_pool.tile([128, 2, 512], FP8, tag="gT", name=f"g8_{g}_{cp}")
                nc.scalar.activation(out=gt8[:], in_=gtp[:], func=AF.Tanh)
                return gt8

            def ct_head(g, alpha):
                pat = ps_ct.tile([64, 8], F32, tag="ct", name=f"pat{g}")
                nc.tensor.transpose(pat[:], alpha[:], ident[:8, :8])
                alpT2 = sm_pool.tile([128, 8], F32, tag="alT2", name=f"aT2{g}")
                nc.vector.tensor_copy(alpT2[0:64, :], pat[:])
                nc.gpsimd.dma_start(out=alpT2[64:128, :], in_=alpT2[0:64, :])
                ars = []
                for rr in range(4):
                    a_r = sm_pool.tile([128, 8], BF16, tag="A", name=f"A{g}_{rr}")
                    nc.gpsimd.tensor_mul(a_r[:], alpT2[:], masks[:, rr, :])
                    ars.append(a_r)
                return ars

            def ct_tail(g, esr_g, ars):
                ct_stage = sm_pool.tile(
                    [8, ED], BF16, tag="ctst", name=f"cts{g}", bufs=1
                )
                for n in range(2):
                    pct = ps_ct.tile([8, 512], F32, tag="ct", name=f"pct{g}_{n}")
                    for rr in range(4):
                        nc.tensor.matmul(
                            pct[:],
                            ars[rr][:],
                            esr_g[:, rr, 512 * n:512 * (n + 1)],
                            start=(rr == 0),
                            stop=(rr == 3),
                        )
                    nc.vector.tensor_copy(ct_stage[:, 512 * n:512 * (n + 1)], pct[:])
                nc.gpsimd.dma_start(out=ct_sb[8 * g:8 * (g + 1), :], in_=ct_stage[:])

            def gh_block(n, ch0, ch1):
                # gh[:, n] = h @ rk[:, n-slice] + bias[n-slice]
                pg = ps_acc.tile([Bc, 512], F32, tag="acc", name=f"pg{n}")
                for d in range(8):
                    ch = ch0 if d < 4 else ch1
                    nc.tensor.matmul(
                        pg[:], hT_sb[:, d, :], ch[:, d % 4, :],
                        start=(d == 0), stop=(d == 7),
                    )
                nc.vector.scalar_tensor_tensor(
                    out=gh_sb[:, 512 * n:512 * (n + 1)],
                    in0=pg[:],
                    scalar=1.0,
                    in1=bias_b[:, 512 * n:512 * (n + 1)],
                    op0=AluOpType.mult,
                    op1=AluOpType.add,
                )

            def gx0_block(n, chunk):
                # inputs-part of the x @ kernel gates
                pa = ps_acc.tile([Bc, 512], F32, tag="acc", name=f"gx0_{n}")
                for d in range(4):
                    nc.tensor.matmul(
                        pa[:], inT_sb[:, d, :], chunk[:, d, :],
                        start=(d == 0), stop=(d == 3),
                    )
                nc.vector.tensor_copy(gx0_sb[:, 512 * n:512 * (n + 1)], pa[:])

            def load_gh_chunk(n, half):
                ch = ghx_pool.tile([128, 4, 512], BF16, tag="ghx", name=f"rkc{n}_{half}")
                src = rk_d.ap().rearrange("(d p) u -> p d u", p=128)
                nc.sync.dma_start(
                    out=ch[:],
                    in_=src[:, 4 * half:4 * (half + 1), 512 * n:512 * (n + 1)],
                )
                return ch

            def load_gx0_chunk(n):
                ch = ghx_pool.tile([128, 4, 512], BF16, tag="ghx", name=f"knc{n}")
                src = kernel_d.ap().rearrange("(d p) u -> p d u", p=128)
                nc.sync.dma_start(
                    out=ch[:], in_=src[:, 0:4, 512 * n:512 * (n + 1)]
                )
                return ch

            def load_rkh(h0, h1):
                src = rk_d.ap().rearrange("(d p) u -> p d u", p=128)
                nc.scalar.dma_start(out=rkh_sb[:, h0:h1], in_=src[:, h0:h1, 2 * U:])

            def emit_pe_softmax(g, gt8s):
                pe = ps_e.tile([1, 512], F32, tag="e", name=f"pe{g}")
                for cp in range(4):
                    nc.tensor.matmul(
                        pe[:], va8_sb[:, :, cp:cp + 1], gt8s[cp][:],
                        start=(cp == 0), stop=(cp == 3),
                        perf_mode=mybir.MatmulPerfMode.DoubleRow,
                    )
                return emit_softmax(g, pe)

            # softmax over t (|e| <~ 1.5: exp w/o max-sub is safe in fp32)
            def emit_softmax(g, pe):
                e_sb = sm_pool.tile([1, 512], F32, tag="esb", name=f"esb{g}", bufs=1)
                nc.vector.tensor_copy(e_sb[:], pe[:])
                alpha = sm_pool.tile([8, T], F32, tag="al", name=f"al{g}")
                nc.gpsimd.dma_start(
                    out=alpha[:],
                    in_=e_sb[0:1, :].rearrange("p (b t) -> p b t", b=8),
                )
                ssum = sm_pool.tile([8, 1], F32, tag="ssum", name=f"ss{g}")
                nc.scalar.activation(
                    out=alpha[:], in_=alpha[:], func=AF.Exp,
                    scale=1.0 / VA_SCALE, accum_out=ssum[:]
                )
                srec = sm_pool.tile([8, 1], F32, tag="srec", name=f"sr{g}")
                nc.vector.reciprocal(srec[:], ssum[:])
                nc.gpsimd.tensor_scalar_mul(alpha[:], alpha[:], srec[:])
                nc.gpsimd.dma_start(
                    out=out_d.ap()[8 * g:8 * (g + 1), 0:T], in_=alpha[:]
                )
                return alpha

            # =================== superblock 0 (j-outer pv) ===================
            tesT = emit_tesT(0, esr_cur)

            # all eight psum banks accumulate u-chunks 0..7 across the j
            # stream so the PE can start as soon as the first wab chunk lands
            pv_banks = [
                ps_v.tile([128, 512], F32, tag="v", name="pvv0"),
                ps_v.tile([128, 512], F32, tag="v", name="pvv1"),
                ps_acc.tile([128, 512], F32, tag="acc", name="pva0"),
                ps_acc.tile([128, 512], F32, tag="acc", name="pva1"),
                ps_e.tile([128, 512], F32, tag="e", name="pve"),
                ps_ct.tile([128, 512], F32, tag="ct", name="pvc"),
                ps_tr.tile([128, 512], F32, tag="tr", name="pvt0"),
                ps_tr.tile([128, 512], F32, tag="tr", name="pvt1"),
            ]
            for jp in range(4):
                for c in range(8):
                    nc.tensor.matmul(
                        pv_banks[c][:],
                        wa8_sb[:, jp, :, 128 * c:128 * (c + 1)],
                        tesT[jp][:],
                        start=(jp == 0),
                        stop=(jp == 3),
                        perf_mode=mybir.MatmulPerfMode.DoubleRow,
                    )
            # raw (pre-qk) gate values to SBUF pairs; frees all banks quickly
            gtps = []
            for cp in range(4):
                gtp = sm_pool.tile([128, 2, 512], BF16, tag="gtp", name=f"gtp0_{cp}")
                gtps.append(gtp)
            for c in range(8):
                nc.vector.tensor_copy(gtps[c // 2][:, c % 2, :], pv_banks[c][:])

            # next sblk's transposes early (tr banks now free)
            tesT_nxt = emit_tesT(1, esr_nxt)

            # qk = tanh(h) @ Wa_top
            pqs = [
                ps_tr.tile([Bc, 512], F32, tag="tr", name=f"pq{hh}")
                for hh in range(2)
            ]
            for d in range(8):
                wat = wat_tiles[d]
                for half in range(2):
                    nc.tensor.matmul(
                        pqs[half][:],
                        thT[:, d, :],
                        wat[:, 512 * half:512 * (half + 1)],
                        start=(d == 0),
                        stop=(d == 7),
                    )
            for half in range(2):
                nc.vector.tensor_copy(qk_nat[:, 512 * half:512 * (half + 1)], pqs[half][:])
            for j in range(8):
                transpose_to(qkT, qk_nat[:, 128 * j:128 * (j + 1)], j, ident, F32)

            # add qk in place (sbuf), then fused pair tanh -> fp8
            for c in range(8):
                qk_slice = qkT[:, c, 0:8]
                qk_bc = bass.AP(
                    tensor=qk_slice.tensor,
                    offset=qk_slice.offset,
                    ap=[list(qk_slice.ap[0]), list(qk_slice.ap[1]), [0, T]],
                )
                gtp = gtps[c // 2]
                nc.vector.scalar_tensor_tensor(
                    out=gtp[:, c % 2, :], in0=gtp[:, c % 2, :], scalar=1.0,
                    in1=qk_bc, op0=AluOpType.mult, op1=AluOpType.add,
                )
            gt8s = [emit_gt8(0, cp, gtps[cp]) for cp in range(4)]

            prev = (0, esr_cur, gt8s)
            esr_cur = esr_nxt
            esr_nxt = None  # loaded inside the loop

            # =================== superblocks 1..7 ===================
            gh_chunks = (load_gh_chunk(0, 0), load_gh_chunk(0, 1))
            for g in range(1, NSBLK):
                tesT = tesT_nxt if g == 1 else emit_tesT(g, esr_cur)

                ars_prev = None
                alpha_prev = None
                gt8s = []
                gtp = None
                for c in range(8):
                    pv = ps_v.tile([128, 512], F32, tag="v", name=f"pv{g}_{c}")
                    for jp in range(4):
                        nc.tensor.matmul(
                            pv[:],
                            wa8_sb[:, jp, :, 128 * c:128 * (c + 1)],
                            tesT[jp][:],
                            start=(jp == 0),
                            stop=(jp == 3),
                            perf_mode=mybir.MatmulPerfMode.DoubleRow,
                        )
                    if c % 2 == 0:
                        gtp = sm_pool.tile(
                            [128, 2, 512], BF16, tag="gtp", name=f"gtp{g}_{c // 2}"
                        )
                    emit_stt_gt(g, c, pv, gtp)
                    if c % 2 == 1:
                        gt8s.append(emit_gt8(g, c // 2, gtp))
                    if c == 1:
                        alpha_prev = emit_pe_softmax(prev[0], prev[2])
                    if c == 4:
                        ars_prev = ct_head(prev[0], alpha_prev)
                    if c == 6:
                        ct_tail(prev[0], prev[1], ars_prev)

                # spread GRU weight streams across the attention phase
                if 1 <= g <= 4:
                    gh_block(g - 1, *gh_chunks)
                    if g < 4:
                        gh_chunks = (load_gh_chunk(g, 0), load_gh_chunk(g, 1))
                    else:
                        gx0_chunks = (load_gx0_chunk(0), load_gx0_chunk(1))
                if 1 <= g <= 4:
                    load_kbot(2 * (g - 1), 2 * g)
                if g == 5:
                    load_rkh(0, 4)
                if g == 6:
                    load_rkh(4, 8)
                if 5 <= g <= 7:
                    n0 = 2 * (g - 5)
                    gx0_block(n0, gx0_chunks[0])
                    gx0_block(n0 + 1, gx0_chunks[1])
                    if g < 7:
                        gx0_chunks = (load_gx0_chunk(n0 + 2), load_gx0_chunk(n0 + 3))

                if g == 7:
                    # pre-sum the z/r and hh additive terms for the tail
                    gzr_sb = sg.tile([Bc, 2 * U], BF16)
                    for n in range(4):
                        nc.vector.tensor_add(
                            gzr_sb[:, 512 * n:512 * (n + 1)],
                            gx0_sb[:, 512 * n:512 * (n + 1)],
                            gh_sb[:, 512 * n:512 * (n + 1)],
                        )
                    hsum_sb = sg.tile([Bc, U], BF16)
                    for n in range(2):
                        nc.vector.tensor_add(
                            hsum_sb[:, 512 * n:512 * (n + 1)],
                            gx0_sb[:, 2 * U + 512 * n:2 * U + 512 * (n + 1)],
                            bias_b[:, 2 * U + 512 * n:2 * U + 512 * (n + 1)],
                        )

                if g + 1 < NSBLK:
                    esr_nxt2 = load_esr(g + 1)
                prev = (g, esr_cur, gt8s)
                esr_cur = esr_nxt2 if g + 1 < NSBLK else None

            # last sblk's pe/softmax + c_t, then the GRU tail
            alpha_last = emit_pe_softmax(prev[0], prev[2])
            ars_prev = ct_head(prev[0], alpha_last)
            ct_tail(prev[0], prev[1], ars_prev)

            # PE warm-keeper: dependency-free matmuls hold the PE pstate up
            # while the last alpha -> c_t -> ctT chain runs on other engines
            for w in range(2):
                pwarm = ps_v.tile([128, 512], F32, tag="v", name=f"warm{w}")
                for k in range(20):
                    nc.tensor.matmul(
                        pwarm[:],
                        identB[:],
                        kbot_sb[:, 0, 0:512],
                        start=(k == 0),
                        stop=(k == 19),
                    )

            # =================== GRU tail ===================
            ctT = sg.tile([128, 8, Bc], BF16)
            for j in range(8):
                transpose_to(ctT, ct_sb[:, 128 * j:128 * (j + 1)], j, identB, BF16)

            z_sb = sg.tile([Bc, U], F32)
            r_sb = sg.tile([Bc, U], F32, tag="scr_r_t1")
            hh_sb = sg.tile([Bc, U], F32)
            rh_sb = sg.tile([Bc, U], BF16)
            rhT = sg.tile([128, 8, Bc], BF16)
            t1 = None  # allocated after r is consumed (shares r's buffer)

            # six gate accumulators in the (now idle) attention psum banks
            gx = [
                ps_tr.tile([Bc, 512], F32, tag="tr", name="gxa"),
                ps_tr.tile([Bc, 512], F32, tag="tr", name="gxb"),
                ps_acc.tile([Bc, 512], F32, tag="acc", name="gxc"),
                ps_acc.tile([Bc, 512], F32, tag="acc", name="gxd"),
                ps_e.tile([Bc, 512], F32, tag="e", name="gxe"),
                ps_ct.tile([Bc, 512], F32, tag="ct", name="gxf"),
            ]
            # pass 1: z/r gate columns (n 0..3) so the gate math starts early
            for d in range(8):
                for n in range(4):
                    nc.tensor.matmul(
                        gx[n][:],
                        ctT[:, d, :],
                        kbot_sb[:, d, 512 * n:512 * (n + 1)],
                        start=(d == 0),
                        stop=(d == 7),
                    )
            # pass 2: hh gate columns (n 4,5); kept open for the rkh stream
            for d in range(8):
                for n in range(4, 6):
                    nc.tensor.matmul(
                        gx[n][:],
                        ctT[:, d, :],
                        kbot_sb[:, d, 512 * n:512 * (n + 1)],
                        start=(d == 0),
                        stop=False,
                    )

            def add_inplace(pa, src_sb, o):
                nc.vector.scalar_tensor_tensor(
                    out=pa[:],
                    in0=pa[:],
                    scalar=1.0,
                    in1=src_sb[:, o:o + 512],
                    op0=AluOpType.mult,
                    op1=AluOpType.add,
                )

            # z, r gates: hard_sigmoid(x) = min(relu(0.2x+0.5), 1)
            for n in range(4):
                dst = z_sb if n < 2 else r_sb
                o = 512 * (n % 2)
                add_inplace(gx[n], gzr_sb, 512 * n)
                nc.scalar.activation(
                    out=dst[:, o:o + 512], in_=gx[n][:],
                    func=AF.Relu, bias=half_sb[:], scale=0.2,
                )
                nc.vector.tensor_scalar_min(dst[:, o:o + 512], dst[:, o:o + 512], 1.0)
                if n >= 2:
                    nc.vector.tensor_mul(
                        rh_sb[:, o:o + 512], r_sb[:, o:o + 512], h_sb[:, o:o + 512]
                    )
                    for j in range(4 * (n - 2), 4 * (n - 1)):
                        transpose_to(rhT, rh_sb[:, 128 * j:128 * (j + 1)], j, identB, BF16)

            # hh accumulators continue with the (r*h) @ rk_hh stream
            for d in range(8):
                for n2 in range(2):
                    nc.tensor.matmul(
                        gx[4 + n2][:],
                        rhT[:, d, :],
                        rkh_sb[:, d, 512 * n2:512 * (n2 + 1)],
                        start=False,
                        stop=(d == 7),
                    )

            # hh = tanh(gates_hh + hsum); h_new = hh + z*(h - hh)
            t1 = sg.tile([Bc, U], F32, tag="scr_r_t1")
            for n2 in range(2):
                o = 512 * n2
                sl = slice(o, o + 512)
                pa = gx[4 + n2]
                add_inplace(pa, hsum_sb, o)
                nc.scalar.activation(out=hh_sb[:, sl], in_=pa[:], func=AF.Tanh)
                nc.vector.tensor_sub(t1[:, sl], h_sb[:, sl], hh_sb[:, sl])
                nc.vector.tensor_mul(t1[:, sl], z_sb[:, sl], t1[:, sl])
                nc.vector.tensor_add(t1[:, sl], hh_sb[:, sl], t1[:, sl])
                nc.sync.dma_start(out=out_d.ap()[:, T + o:T + o + 512], in_=t1[:, sl])

    return nc


_built = [None]


def _to_bf16(x):
    return np.ascontiguousarray(np.asarray(x, dtype=np.float32)).astype(NP_BF16)


def _transpose_chunks(x, nd):
    # x [Bc, nd*128] f32 -> [128, nd*Bc] bf16 with out[p, d*Bc+b] = x[b, 128d+p]
    b, _ = x.shape
    xt = np.ascontiguousarray(x.T).reshape(nd, 128, b).transpose(1, 0, 2)
    return np.ascontiguousarray(xt.reshape(128, nd * b)).astype(NP_BF16)


def make_in_maps(inputs):
    def f32(name):
        return np.ascontiguousarray(np.asarray(inputs[name], dtype=np.float32))

    inp = f32("inputs")
    h = f32("h")
    es = f32("encoder_states")
    ker_b = _to_bf16(inputs["kernel"])
    rk_b = _to_bf16(inputs["recurrent_kernel"])
    bias_b = _to_bf16(inputs["bias"])
    wa = np.ascontiguousarray(np.asarray(inputs["Wa"], dtype=np.float32))
    wat_b = wa[:U].astype(NP_BF16)
    wa8 = np.ascontiguousarray(
        wa[U:].reshape(4, 2, 128, U).transpose(2, 0, 1, 3)
    ).astype(NP_FP8)
    va = np.asarray(inputs["Va"], dtype=np.float32)
    va_b = np.ascontiguousarray(va[:, 0].reshape(8, 128).T).astype(NP_BF16)
    va8 = np.zeros((128, 2, 16), dtype=NP_FP8)
    va8[:, :, 0:4] = np.ascontiguousarray(
        (VA_SCALE * va[:, 0]).reshape(4, 2, 128).transpose(2, 1, 0)
    ).astype(NP_FP8)

    in_maps = []
    for c in range(N_CORES):
        sl = slice(c * Bc, (c + 1) * Bc)
        sp = np.concatenate(
            [_transpose_chunks(h[sl], 8), _transpose_chunks(inp[sl], 4), va_b],
            axis=1,
        )
        in_maps.append({
            "esr": _to_bf16(es[sl].reshape(Bc * T, ED)),
            "h": h[sl],
            "smallpack": np.ascontiguousarray(sp),
            "kernel": ker_b,
            "recurrent_kernel": rk_b,
            "bias": bias_b,
            "wat": wat_b,
            "wa8": wa8,
            "va8": va8,
        })
    return in_maps


def kernel(**inputs):
    if _built[0] is None:
        nc = build_nc()
        fix_multi_waits(nc)
        _built[0] = nc
    nc = _built[0]

    from concourse.bass_utils import run_bass_kernel_spmd

    in_maps = make_in_maps(inputs)
    res = run_bass_kernel_spmd(nc, in_maps, list(range(N_CORES)))
    out = np.concatenate(
        [res.results[c]["out"] for c in range(N_CORES)], axis=0
    ).astype(np.float32)
    return out


# revision 30
# speedup vs baseline: 1.1089x; 1.0302x over previous
"""Self-contained Trainium2 (Bass/Tile) kernel for the AttentionGRUCell
problem: 8-core data-parallel over batch, bf16 matmul operands
(host-cast), fp32 accumulation.

kernel(**inputs) takes the FULL unsharded inputs and returns the FULL
[512, 1088] output ([alpha, h_new] per row), running the Bass program on
NeuronCores 0-7 via run_bass_kernel_spmd.
"""
import sys

for _p in ("/opt/trn_rl_repo",):
    if _p not in sys.path:
        sys.path.insert(0, _p)

import numpy as np
import ml_dtypes
import concourse.bass as bass
import concourse.mybir as mybir
import concourse.tile as tile
import bass_rust
from concourse.alu_op_type import AluOpType
from concourse.masks import make_identity
from concourse.vector_clock import ScopedClock

F32 = mybir.dt.float32
BF16 = mybir.dt.bfloat16
FP8 = mybir.dt.float8e4
AF = mybir.ActivationFunctionType
AX = mybir.AxisListType

Bc, T, XD, ED, U = 64, 64, 512, 1024, 1024
NSBLK = 8
N_CORES = 8
B_FULL = 512

NP_BF16 = ml_dtypes.bfloat16
NP_FP8 = ml_dtypes.float8_e4m3
VA_SCALE = 32.0


# ---------------------------------------------------------------------------
# Workarounds for this walrus build: instructions may carry at most one sem
# wait ("Too many sync wait commands"), including the Tile kernel-tail drain.
# ---------------------------------------------------------------------------

def _patched_drain_and_barrier(self, tick_clock, wait_clock):
    nc = self.nc
    probe = nc.sync.nop(nofuse=True)
    wait_clock.add_sem_waits(probe.ins, ScopedClock({None: tick_clock.global_clock}))
    si = probe.ins.sync_info
    waits = list(si.on_wait) if si is not None else []
    probe.ins.sync_info = bass_rust.SyncInfo(on_wait=waits[:1], on_update=[])
    for w in waits[1:]:
        n2 = nc.sync.nop(nofuse=True)
        n2.ins.sync_info = bass_rust.SyncInfo(on_wait=[w], on_update=[])
    nc.sync.drain()
    nc.all_engine_barrier()
    assert self.sems is not None
    popped = nc._tile_sem_poison_stack.pop()
    assert popped is self._sem_poison
    nc.clear_and_free_semaphores(list(self.sems.allocated().values()))
    nc.all_engine_barrier()


tile.TileContext._drain_and_barrier = _patched_drain_and_barrier

_fix_ctr = [0]


def fix_multi_waits(nc, max_waits=1):
    """Hoist extra sem waits onto same-engine InstNoOps placed immediately
    before the instruction -- engines execute in order, so semantics are
    identical."""
    for f in nc.m.functions:
        for blk in f.blocks:
            insts = blk.instructions
            if not any(
                i.sync_info is not None and len(i.sync_info.on_wait) > max_waits
                for i in insts
            ):
                continue
            out = []
            for inst in insts:
                si = inst.sync_info
                if si is not None and len(si.on_wait) > max_waits:
                    waits = list(si.on_wait)
                    for w in waits[:-max_waits]:
                        _fix_ctr[0] += 1
                        nop = mybir.InstNoOp(
                            name=f"waitfix-{_fix_ctr[0]}",
                            ins=[],
                            outs=[],
                            engine=inst.engine,
                        )
                        nop.sync_info = bass_rust.SyncInfo(on_wait=[w], on_update=[])
                        out.append(nop)
                    inst.sync_info = bass_rust.SyncInfo(
                        on_wait=waits[-max_waits:], on_update=list(si.on_update)
                    )
                out.append(inst)
            blk.instructions = out


# ---------------------------------------------------------------------------
# Kernel program
# ---------------------------------------------------------------------------

def build_nc():
    nc = bass.Bass("TRN2", target_bir_lowering=False, debug=False)

    # host-prepped per-core inputs (all bf16 except h/bias):
    #   esr:  [Bc*T, ED]      natural layout, row = 64*b + t
    #   hT:   [128, 8*Bc]     hT[p, d*Bc+b] = h[b, 128d+p]
    #   inT:  [128, 4*Bc]     inT[p, d*Bc+b] = inputs[b, 128d+p]
    #   va:   [128, 8]        va[p, j] = Va[128j+p, 0]
    es_d = nc.dram_tensor("esr", [Bc * T, ED], BF16, kind="ExternalInput")
    h_d = nc.dram_tensor("h", [Bc, U], F32, kind="ExternalInput")
    # smallpack: hT [128, 512] | inT [128, 256] | va [128, 8]  (one DMA)
    sp_d = nc.dram_tensor("smallpack", [128, 8 * Bc + 4 * Bc + 8 + 128], BF16,
                          kind="ExternalInput")
    kernel_d = nc.dram_tensor("kernel", [XD + ED, 3 * U], BF16, kind="ExternalInput")
    rk_d = nc.dram_tensor("recurrent_kernel", [U, 3 * U], BF16, kind="ExternalInput")
    bias_d = nc.dram_tensor("bias", [3 * U], BF16, kind="ExternalInput")
    wat_d = nc.dram_tensor("wat", [U, U], BF16, kind="ExternalInput")
    wa8_d = nc.dram_tensor("wa8", [128, 4, 2, U], FP8, kind="ExternalInput")
    va8_d = nc.dram_tensor("va8", [128, 2, 16], FP8, kind="ExternalInput")
    out_d = nc.dram_tensor("out", [Bc, T + U], F32, kind="ExternalOutput")

    es_ap = es_d.ap()

    with tile.TileContext(nc) as tc:
        with (
            tc.tile_pool(name="singles", bufs=1) as sg,
            tc.tile_pool(name="esr", bufs=3) as esr_pool,
            tc.tile_pool(name="tesT", bufs=8) as tesT_pool,
            tc.tile_pool(name="gT", bufs=8) as gT_pool,
            tc.tile_pool(name="wat", bufs=8) as wat_pool,
            tc.tile_pool(name="ghx", bufs=2) as ghx_pool,
            tc.tile_pool(name="smalls", bufs=4) as sm_pool,
            # 8 PSUM banks total; tail reuses attention tags
            tc.tile_pool(name="ps_tr", bufs=2, space="PSUM") as ps_tr,
            tc.tile_pool(name="ps_v", bufs=2, space="PSUM") as ps_v,
            tc.tile_pool(name="ps_acc", bufs=2, space="PSUM") as ps_acc,
            tc.tile_pool(name="ps_e", bufs=1, space="PSUM") as ps_e,
            tc.tile_pool(name="ps_ct", bufs=1, space="PSUM") as ps_ct,
        ):
            # ---- earliest DMAs: es sblk0, weights ----
            def load_esr(g, eng=None):
                e_t = esr_pool.tile([128, 4, ED], BF16, tag="esr", name=f"esr{g}")
                src = es_ap.rearrange("(g r p) e -> g p r e", p=128, r=4)
                e = eng or nc.gpsimd
                e.dma_start(out=e_t[:, 0:2, :], in_=src[g, :, 0:2, :])
                e.dma_start(out=e_t[:, 2:4, :], in_=src[g, :, 2:4, :])
                return e_t

            # identity rides in smallpack (DMA), so the PE warm-up can start
            # as soon as that small transfer lands
            sp_sb = sg.tile([128, 8 * Bc + 4 * Bc + 8 + 128], BF16)
            nc.scalar.dma_start(out=sp_sb[:], in_=sp_d.ap())
            identB = sp_sb[:, 12 * Bc + 8:12 * Bc + 8 + 128]
            ident = sg.tile([128, 128], F32)
            nc.vector.tensor_copy(ident[:], identB)

            esr_cur = load_esr(0, eng=nc.sync)

            # PE warm-up: dependency-free transposes ramp the PE pstate while
            # the first es/weight DMAs land
            for w in range(60):
                pw = ps_tr.tile([128, 128], BF16, tag="tr", name=f"wu{w}")
                nc.tensor.transpose(pw[:], identB, identB)

            wa8_sb = sg.tile([128, 4, 2, U], FP8)
            nc.sync.dma_start(out=wa8_sb[:, 0:2], in_=wa8_d.ap()[:, 0:2])
            nc.sync.dma_start(out=wa8_sb[:, 2:4], in_=wa8_d.ap()[:, 2:4])
            va8_sb = sg.tile([128, 2, 16], FP8)
            nc.sync.dma_start(out=va8_sb[:], in_=va8_d.ap())

            hT_sb = sp_sb[:, 0:8 * Bc].rearrange("p (d b) -> p d b", b=Bc)
            inT_sb = sp_sb[:, 8 * Bc:12 * Bc].rearrange("p (d b) -> p d b", b=Bc)
            va_sb = sp_sb[:, 12 * Bc:12 * Bc + 8]
            h_sb = sg.tile([Bc, U], F32)
            nc.scalar.dma_start(out=h_sb[:], in_=h_d[:])
            bias_b = sg.tile([Bc, 3 * U], BF16)
            bias_src = bias_d.ap()
            nc.scalar.dma_start(
                out=bias_b[:],
                in_=bass.AP(
                    tensor=bias_src.tensor,
                    offset=bias_src.offset,
                    ap=[[0, Bc], list(bias_src.ap[0])],
                ),
            )

            # GRU kernel bottom (c_t part): DMA emitted at g==1
            kbot_sb = sg.tile([128, 8, 3 * U], BF16)

            def load_kbot(d0, d1):
                src = kernel_d.ap()[XD:, :].rearrange("(d p) u -> p d u", p=128)
                nc.gpsimd.dma_start(out=kbot_sb[:, d0:d1], in_=src[:, d0:d1])

            # Wa top half (h part), for qk -- all 8 chunks up front (scalar q)
            def load_wat(d):
                w_t = wat_pool.tile([128, U], BF16, tag="wat", name=f"wat{d}")
                nc.sync.dma_start(out=w_t[:], in_=wat_d.ap()[128 * d:128 * (d + 1), :])
                return w_t

            wat_tiles = [load_wat(d) for d in range(8)]

            esr_nxt = load_esr(1, eng=nc.sync)

            # thT = tanh(h).T directly in transposed layout
            thT = sg.tile([128, 8, Bc], BF16)
            nc.scalar.activation(out=thT[:], in_=hT_sb, func=AF.Tanh)

            # masks for block-diag A build
            masks = sg.tile([128, 4, 8], F32)
            nc.vector.memset(masks[:], 0.0)
            for rr in range(4):
                nc.vector.memset(masks[0:64, rr, 2 * rr:2 * rr + 1], 1.0)
                nc.vector.memset(masks[64:128, rr, 2 * rr + 1:2 * rr + 2], 1.0)

            half_sb = sg.tile([Bc, 1], F32)
            nc.vector.memset(half_sb[:], 0.5)

            ct_sb = sg.tile([Bc, ED], BF16)
            gh_sb = sg.tile([Bc, 2 * U], BF16)
            gx0_sb = sg.tile([Bc, 3 * U], BF16)
            qk_nat = sg.tile([Bc, U], F32)
            qkT = sg.tile([128, 8, Bc], BF16)

            # GRU recurrent_kernel hh columns (DMA emitted mid-loop)
            rkh_sb = sg.tile([128, 8, U], BF16)

            _tp_ctr = [0]

            def transpose_to(dst, src_2d, j, idt, dt):
                _tp_ctr[0] += 1
                pt = ps_tr.tile([128, Bc], dt, tag="tr", name=f"tp{_tp_ctr[0]}")
                nc.tensor.transpose(pt[:], src_2d, idt[:Bc, :Bc])
                nc.vector.tensor_copy(dst[:, j, :], pt[:])

            # ---- per-sblk building blocks ----
            def emit_tesT(g, esr_g):
                # DoubleRow pairs: tesT[jp][p, s, bt] = tanh(es.T)[256jp+128s+p, bt]
                tesT = []
                for jp in range(4):
                    pt = ps_tr.tile([128, 2, 512], BF16, tag="tr", name=f"ptr{g}_{jp}")
                    for jj in range(2):
                        for rr in range(4):
                            nc.tensor.transpose(
                                pt[:, jj, 128 * rr:128 * (rr + 1)],
                                esr_g[:, rr, 128 * (2 * jp + jj):128 * (2 * jp + jj + 1)],
                                identB[:],
                            )
                    tt = tesT_pool.tile([128, 2, 512], FP8, tag="tesT", name=f"tes{g}_{jp}")
                    nc.scalar.activation(out=tt[:], in_=pt[:], func=AF.Tanh)
                    tesT.append(tt)
                return tesT

            def emit_stt_gt(g, c, pv, gtp):
                # gtp[:, c%2, :] = pv + qk (broadcast over t)
                qk_slice = qkT[:, c, 8 * g:8 * g + 8]
                qk_bc = bass.AP(
                    tensor=qk_slice.tensor,
                    offset=qk_slice.offset,
                    ap=[
                        list(qk_slice.ap[0]),
                        list(qk_slice.ap[1]),
                        [0, T],
                    ],
                )
                nc.vector.scalar_tensor_tensor(
                    out=gtp[:, c % 2, :],
                    in0=pv[:],
                    scalar=1.0,
                    in1=qk_bc,
                    op0=AluOpType.mult,
                    op1=AluOpType.add,
                )

            def emit_gt8(g, cp, gtp):
                gt8 = gT_pool.tile([128, 2, 512], FP8, tag="gT", name=f"g8_{g}_{cp}")
                nc.scalar.activation(out=gt8[:], in_=gtp[:], func=AF.Tanh)
                return gt8

            def ct_head(g, alpha):
                pat = ps_ct.tile([64, 8], F32, tag="ct", name=f"pat{g}")
                nc.tensor.transpose(pat[:], alpha[:], ident[:8, :8])
                alpT2 = sm_pool.tile([128, 8], F32, tag="alT2", name=f"aT2{g}")
                nc.vector.tensor_copy(alpT2[0:64, :], pat[:])
                nc.gpsimd.dma_start(out=alpT2[64:128, :], in_=alpT2[0:64, :])
                ars = []
                for rr in range(4):
                    a_r = sm_pool.tile([128, 8], BF16, tag="A", name=f"A{g}_{rr}")
                    nc.gpsimd.tensor_mul(a_r[:], alpT2[:], masks[:, rr, :])
                    ars.append(a_r)
                return ars

            def ct_tail(g, esr_g, ars):
                ct_stage = sm_pool.tile(
                    [8, ED], BF16, tag="ctst", name=f"cts{g}", bufs=1
                )
                for n in range(2):
                    pct = ps_ct.tile([8, 512], F32, tag="ct", name=f"pct{g}_{n}")
                    for rr in range(4):
                        nc.tensor.matmul(
                            pct[:],
                            ars[rr][:],
                            esr_g[:, rr, 512 * n:512 * (n + 1)],
                            start=(rr == 0),
                            stop=(rr == 3),
                        )
                    nc.vector.tensor_copy(ct_stage[:, 512 * n:512 * (n + 1)], pct[:])
                nc.gpsimd.dma_start(out=ct_sb[8 * g:8 * (g + 1), :], in_=ct_stage[:])

            def gh_block(n, ch0, ch1):
                # gh[:, n] = h @ rk[:, n-slice] + bias[n-slice]
                pg = ps_acc.tile([Bc, 512], F32, tag="acc", name=f"pg{n}")
                for d in range(8):
                    ch = ch0 if d < 4 else ch1
                    nc.tensor.matmul(
                        pg[:], hT_sb[:, d, :], ch[:, d % 4, :],
                        start=(d == 0), stop=(d == 7),
                    )
                nc.vector.scalar_tensor_tensor(
                    out=gh_sb[:, 512 * n:512 * (n + 1)],
                    in0=pg[:],
                    scalar=1.0,
                    in1=bias_b[:, 512 * n:512 * (n + 1)],
                    op0=AluOpType.mult,
                    op1=AluOpType.add,
                )

            def gx0_block(n, chunk):
                # inputs-part of the x @ kernel gates
                pa = ps_acc.tile([Bc, 512], F32, tag="acc", name=f"gx0_{n}")
                for d in range(4):
                    nc.tensor.matmul(
                        pa[:], inT_sb[:, d, :], chunk[:, d, :],
                        start=(d == 0), stop=(d == 3),
                    )
                nc.vector.tensor_copy(gx0_sb[:, 512 * n:512 * (n + 1)], pa[:])

            def load_gh_chunk(n, half):
                ch = ghx_pool.tile([128, 4, 512], BF16, tag="ghx", name=f"rkc{n}_{half}")
                src = rk_d.ap().rearrange("(d p) u -> p d u", p=128)
                nc.sync.dma_start(
                    out=ch[:],
                    in_=src[:, 4 * half:4 * (half + 1), 512 * n:512 * (n + 1)],
                )
                return ch

            def load_gx0_chunk(n):
                ch = ghx_pool.tile([128, 4, 512], BF16, tag="ghx", name=f"knc{n}")
                src = kernel_d.ap().rearrange("(d p) u -> p d u", p=128)
                nc.sync.dma_start(
                    out=ch[:], in_=src[:, 0:4, 512 * n:512 * (n + 1)]
                )
                return ch

            def load_rkh(h0, h1):
                src = rk_d.ap().rearrange("(d p) u -> p d u", p=128)
                nc.gpsimd.dma_start(out=rkh_sb[:, h0:h1], in_=src[:, h0:h1, 2 * U:])

            def emit_pe_softmax(g, gt8s):
                pe = ps_e.tile([1, 512], F32, tag="e", name=f"pe{g}")
                for cp in range(4):
                    nc.tensor.matmul(
                        pe[:], va8_sb[:, :, cp:cp + 1], gt8s[cp][:],
                        start=(cp == 0), stop=(cp == 3),
                        perf_mode=mybir.MatmulPerfMode.DoubleRow,
                    )
                return emit_softmax(g, pe)

            # softmax over t (|e| <~ 1.5: exp w/o max-sub is safe in fp32)
            def emit_softmax(g, pe):
                e_sb = sm_pool.tile([1, 512], F32, tag="esb", name=f"esb{g}", bufs=1)
                nc.vector.tensor_copy(e_sb[:], pe[:])
                alpha = sm_pool.tile([8, T], F32, tag="al", name=f"al{g}")
                nc.gpsimd.dma_start(
                    out=alpha[:],
                    in_=e_sb[0:1, :].rearrange("p (b t) -> p b t", b=8),
                )
                ssum = sm_pool.tile([8, 1], F32, tag="ssum", name=f"ss{g}")
                nc.scalar.activation(
                    out=alpha[:], in_=alpha[:], func=AF.Exp,
                    scale=1.0 / VA_SCALE, accum_out=ssum[:]
                )
                srec = sm_pool.tile([8, 1], F32, tag="srec", name=f"sr{g}")
                nc.vector.reciprocal(srec[:], ssum[:])
                nc.gpsimd.tensor_scalar_mul(alpha[:], alpha[:], srec[:])
                nc.gpsimd.dma_start(
                    out=out_d.ap()[8 * g:8 * (g + 1), 0:T], in_=alpha[:]
                )
                return alpha

            # =================== superblock 0 (j-outer pv) ===================
            tesT = emit_tesT(0, esr_cur)

            # all eight psum banks accumulate u-chunks 0..7 across the j
            # stream so the PE can start as soon as the first wab chunk lands
            pv_banks = [
                ps_v.tile([128, 512], F32, tag="v", name="pvv0"),
                ps_v.tile([128, 512], F32, tag="v", name="pvv1"),
                ps_acc.tile([128, 512], F32, tag="acc", name="pva0"),
                ps_acc.tile([128, 512], F32, tag="acc", name="pva1"),
                ps_e.tile([128, 512], F32, tag="e", name="pve"),
                ps_ct.tile([128, 512], F32, tag="ct", name="pvc"),
                ps_tr.tile([128, 512], F32, tag="tr", name="pvt0"),
                ps_tr.tile([128, 512], F32, tag="tr", name="pvt1"),
            ]
            for jp in range(4):
                for c in range(8):
                    nc.tensor.matmul(
                        pv_banks[c][:],
                        wa8_sb[:, jp, :, 128 * c:128 * (c + 1)],
                        tesT[jp][:],
                        start=(jp == 0),
                        stop=(jp == 3),
                        perf_mode=mybir.MatmulPerfMode.DoubleRow,
                    )
            # raw (pre-qk) gate values to SBUF pairs; frees all banks quickly
            gtps = []
            for cp in range(4):
                gtp = sm_pool.tile([128, 2, 512], BF16, tag="gtp", name=f"gtp0_{cp}")
                gtps.append(gtp)
            for c in range(8):
                nc.vector.tensor_copy(gtps[c // 2][:, c % 2, :], pv_banks[c][:])

            # qk = tanh(h) @ Wa_top
            pqs = [
                ps_tr.tile([Bc, 512], F32, tag="tr", name=f"pq{hh}")
                for hh in range(2)
            ]
            for d in range(8):
                wat = wat_tiles[d]
                for half in range(2):
                    nc.tensor.matmul(
                        pqs[half][:],
                        thT[:, d, :],
                        wat[:, 512 * half:512 * (half + 1)],
                        start=(d == 0),
                        stop=(d == 7),
                    )
            for half in range(2):
                nc.vector.tensor_copy(qk_nat[:, 512 * half:512 * (half + 1)], pqs[half][:])
            for j in range(8):
                transpose_to(qkT, qk_nat[:, 128 * j:128 * (j + 1)], j, ident, F32)

            # next sblk's transposes (tr banks free again)
            tesT_nxt = emit_tesT(1, esr_nxt)

            # add qk in place (sbuf), then fused pair tanh -> fp8
            for c in range(8):
                qk_slice = qkT[:, c, 0:8]
                qk_bc = bass.AP(
                    tensor=qk_slice.tensor,
                    offset=qk_slice.offset,
                    ap=[list(qk_slice.ap[0]), list(qk_slice.ap[1]), [0, T]],
                )
                gtp = gtps[c // 2]
                nc.vector.scalar_tensor_tensor(
                    out=gtp[:, c % 2, :], in0=gtp[:, c % 2, :], scalar=1.0,
                    in1=qk_bc, op0=AluOpType.mult, op1=AluOpType.add,
                )
            gt8s = [emit_gt8(0, cp, gtps[cp]) for cp in range(4)]

            prev = (0, esr_cur, gt8s)
            esr_cur = esr_nxt
            esr_nxt = None  # loaded inside the loop

            # =================== superblocks 1..7 ===================
            gh_chunks = (load_gh_chunk(0, 0), load_gh_chunk(0, 1))
            for g in range(1, NSBLK):
                tesT = tesT_nxt if g == 1 else emit_tesT(g, esr_cur)

                ars_prev = None
                alpha_prev = None
                gt8s = []
                gtp = None
                for c in range(8):
                    pv = ps_v.tile([128, 512], F32, tag="v", name=f"pv{g}_{c}")
                    for jp in range(4):
                        nc.tensor.matmul(
                            pv[:],
                            wa8_sb[:, jp, :, 128 * c:128 * (c + 1)],
                            tesT[jp][:],
                            start=(jp == 0),
                            stop=(jp == 3),
                            perf_mode=mybir.MatmulPerfMode.DoubleRow,
                        )
                    if c % 2 == 0:
                        gtp = sm_pool.tile(
                            [128, 2, 512], BF16, tag="gtp", name=f"gtp{g}_{c // 2}"
                        )
                    emit_stt_gt(g, c, pv, gtp)
                    if c % 2 == 1:
                        gt8s.append(emit_gt8(g, c // 2, gtp))
                    if c == 1:
                        alpha_prev = emit_pe_softmax(prev[0], prev[2])
                    if c == 4:
                        ars_prev = ct_head(prev[0], alpha_prev)
                    if c == 6:
                        ct_tail(prev[0], prev[1], ars_prev)

                # spread GRU weight streams across the attention phase
                if 1 <= g <= 4:
                    gh_block(g - 1, *gh_chunks)
                    if g < 4:
                        gh_chunks = (load_gh_chunk(g, 0), load_gh_chunk(g, 1))
                    else:
                        gx0_chunks = (load_gx0_chunk(0), load_gx0_chunk(1))
                if 1 <= g <= 4:
                    load_kbot(2 * (g - 1), 2 * g)
                if g == 5:
                    load_rkh(0, 4)
                if g == 6:
                    load_rkh(4, 8)
                if 5 <= g <= 7:
                    n0 = 2 * (g - 5)
                    gx0_block(n0, gx0_chunks[0])
                    gx0_block(n0 + 1, gx0_chunks[1])
                    if g < 7:
                        gx0_chunks = (load_gx0_chunk(n0 + 2), load_gx0_chunk(n0 + 3))

                if g == 7:
                    # pre-sum the z/r and hh additive terms for the tail
                    gzr_sb = sg.tile([Bc, 2 * U], BF16)
                    for n in range(4):
                        nc.vector.tensor_add(
                            gzr_sb[:, 512 * n:512 * (n + 1)],
                            gx0_sb[:, 512 * n:512 * (n + 1)],
                            gh_sb[:, 512 * n:512 * (n + 1)],
                        )
                    hsum_sb = sg.tile([Bc, U], BF16)
                    for n in range(2):
                        nc.vector.tensor_add(
                            hsum_sb[:, 512 * n:512 * (n + 1)],
                            gx0_sb[:, 2 * U + 512 * n:2 * U + 512 * (n + 1)],
                            bias_b[:, 2 * U + 512 * n:2 * U + 512 * (n + 1)],
                        )

                if g + 1 < NSBLK:
                    esr_nxt2 = load_esr(g + 1)
                prev = (g, esr_cur, gt8s)
                esr_cur = esr_nxt2 if g + 1 < NSBLK else None

            # last sblk's pe/softmax + c_t, then the GRU tail
            alpha_last = emit_pe_softmax(prev[0], prev[2])
            ars_prev = ct_head(prev[0], alpha_last)
            ct_tail(prev[0], prev[1], ars_prev)

            # PE warm-keeper: dependency-free matmuls hold the PE pstate up
            # while the last alpha -> c_t -> ctT chain runs on other engines
            for w in range(2):
                pwarm = ps_v.tile([128, 512], F32, tag="v", name=f"warm{w}")
                for k in range(28):
                    nc.tensor.matmul(
                        pwarm[:],
                        identB,
                        kbot_sb[:, 0, 0:512],
                        start=(k == 0),
                        stop=(k == 27),
                    )

            # =================== GRU tail ===================
            ctT = sg.tile([128, 8, Bc], BF16)
            for j in range(8):
                transpose_to(ctT, ct_sb[:, 128 * j:128 * (j + 1)], j, identB, BF16)

            z_sb = sg.tile([Bc, U], F32)
            r_sb = sg.tile([Bc, U], F32, tag="scr_r_t1")
            hh_sb = sg.tile([Bc, U], F32)
            rh_sb = sg.tile([Bc, U], BF16)
            rhT = sg.tile([128, 8, Bc], BF16)
            t1 = None  # allocated after r is consumed (shares r's buffer)

            # six gate accumulators in the (now idle) attention psum banks
            gx = [
                ps_tr.tile([Bc, 512], F32, tag="tr", name="gxa"),
                ps_tr.tile([Bc, 512], F32, tag="tr", name="gxb"),
                ps_acc.tile([Bc, 512], F32, tag="acc", name="gxc"),
                ps_acc.tile([Bc, 512], F32, tag="acc", name="gxd"),
                ps_e.tile([Bc, 512], F32, tag="e", name="gxe"),
                ps_ct.tile([Bc, 512], F32, tag="ct", name="gxf"),
            ]
            # pass 1: z/r gate columns, n-outer so each gate finishes early
            for n in range(4):
                for d in range(8):
                    nc.tensor.matmul(
                        gx[n][:],
                        ctT[:, d, :],
                        kbot_sb[:, d, 512 * n:512 * (n + 1)],
                        start=(d == 0),
                        stop=(d == 7),
                    )
            # pass 2: hh gate columns (n 4,5); kept open for the rkh stream
            for d in range(8):
                for n in range(4, 6):
                    nc.tensor.matmul(
                        gx[n][:],
                        ctT[:, d, :],
                        kbot_sb[:, d, 512 * n:512 * (n + 1)],
                        start=(d == 0),
                        stop=False,
                    )

            def add_inplace(pa, src_sb, o):
                nc.vector.scalar_tensor_tensor(
                    out=pa[:],
                    in0=pa[:],
                    scalar=1.0,
                    in1=src_sb[:, o:o + 512],
                    op0=AluOpType.mult,
                    op1=AluOpType.add,
                )

            # z, r gates: hard_sigmoid(x) = min(relu(0.2x+0.5), 1)
            for n in range(4):
                dst = z_sb if n < 2 else r_sb
                o = 512 * (n % 2)
                add_inplace(gx[n], gzr_sb, 512 * n)
                nc.scalar.activation(
                    out=dst[:, o:o + 512], in_=gx[n][:],
                    func=AF.Relu, bias=half_sb[:], scale=0.2,
                )
                nc.vector.tensor_scalar_min(dst[:, o:o + 512], dst[:, o:o + 512], 1.0)
                if n >= 2:
                    nc.vector.tensor_mul(
                        rh_sb[:, o:o + 512], r_sb[:, o:o + 512], h_sb[:, o:o + 512]
                    )
                    for j in range(4 * (n - 2), 4 * (n - 1)):
                        transpose_to(rhT, rh_sb[:, 128 * j:128 * (j + 1)], j, identB, BF16)

            # hh accumulators continue with the (r*h) @ rk_hh stream
            for d in range(8):
                for n2 in range(2):
                    nc.tensor.matmul(
                        gx[4 + n2][:],
                        rhT[:, d, :],
                        rkh_sb[:, d, 512 * n2:512 * (n2 + 1)],
                        start=False,
                        stop=(d == 7),
                    )

            # hh = tanh(gates_hh + hsum); h_new = hh + z*(h - hh)
            t1 = sg.tile([Bc, U], F32, tag="scr_r_t1")
            for n2 in range(2):
                o = 512 * n2
                sl = slice(o, o + 512)
                pa = gx[4 + n2]
                add_inplace(pa, hsum_sb, o)
                nc.scalar.activation(out=hh_sb[:, sl], in_=pa[:], func=AF.Tanh)
                nc.vector.tensor_sub(t1[:, sl], h_sb[:, sl], hh_sb[:, sl])
                nc.vector.tensor_mul(t1[:, sl], z_sb[:, sl], t1[:, sl])
                nc.vector.tensor_add(t1[:, sl], hh_sb[:, sl], t1[:, sl])
                nc.sync.dma_start(out=out_d.ap()[:, T + o:T + o + 512], in_=t1[:, sl])

    return nc


_built = [None]


def _to_bf16(x):
    return np.ascontiguousarray(np.asarray(x, dtype=np.float32)).astype(NP_BF16)


def _transpose_chunks(x, nd):
    # x [Bc, nd*128] f32 -> [128, nd*Bc] bf16 with out[p, d*Bc+b] = x[b, 128d+p]
    b, _ = x.shape
    xt = np.ascontiguousarray(x.T).reshape(nd, 128, b).transpose(1, 0, 2)
    return np.ascontiguousarray(xt.reshape(128, nd * b)).astype(NP_BF16)


def make_in_maps(inputs):
    def f32(name):
        return np.ascontiguousarray(np.asarray(inputs[name], dtype=np.float32))

    inp = f32("inputs")
    h = f32("h")
    es = f32("encoder_states")
    ker_b = _to_bf16(inputs["kernel"])
    rk_b = _to_bf16(inputs["recurrent_kernel"])
    bias_b = _to_bf16(inputs["bias"])
    wa = np.ascontiguousarray(np.asarray(inputs["Wa"], dtype=np.float32))
    wat_b = wa[:U].astype(NP_BF16)
    wa8 = np.ascontiguousarray(
        wa[U:].reshape(4, 2, 128, U).transpose(2, 0, 1, 3)
    ).astype(NP_FP8)
    va = np.asarray(inputs["Va"], dtype=np.float32)
    va_b = np.ascontiguousarray(va[:, 0].reshape(8, 128).T).astype(NP_BF16)
    va8 = np.zeros((128, 2, 16), dtype=NP_FP8)
    va8[:, :, 0:4] = np.ascontiguousarray(
        (VA_SCALE * va[:, 0]).reshape(4, 2, 128).transpose(2, 1, 0)
    ).astype(NP_FP8)

    in_maps = []
    for c in range(N_CORES):
        sl = slice(c * Bc, (c + 1) * Bc)
        sp = np.concatenate(
            [_transpose_chunks(h[sl], 8), _transpose_chunks(inp[sl], 4), va_b,
             np.eye(128, dtype=NP_BF16)],
            axis=1,
        )
        in_maps.append({
            "esr": _to_bf16(es[sl].reshape(Bc * T, ED)),
            "h": h[sl],
            "smallpack": np.ascontiguousarray(sp),
            "kernel": ker_b,
            "recurrent_kernel": rk_b,
            "bias": bias_b,
            "wat": wat_b,
            "wa8": wa8,
            "va8": va8,
        })
    return in_maps


def kernel(**inputs):
    if _built[0] is None:
        nc = build_nc()
        fix_multi_waits(nc)
        _built[0] = nc
    nc = _built[0]

    from concourse.bass_utils import run_bass_kernel_spmd

    in_maps = make_in_maps(inputs)
    res = run_bass_kernel_spmd(nc, in_maps, list(range(N_CORES)))
    out = np.concatenate(
        [res.results[c]["out"] for c in range(N_CORES)], axis=0
    ).astype(np.float32)
    return out
